# revision 1
# baseline (speedup 1.0000x reference)
"""Trainium2 Bass kernel for batched global-sum attention (B=8, C=256, N=2048).

Math (per sample b, one NeuronCore each — batch is sharded across 8 cores):
    q = Wq x + bq 1^T ; k = Wk x + bk 1^T ; v = Wv x + bv 1^T        (x: [C,N])
    qk = q^T k ;  attn = v (qk / S) ,  S = sum_b sum(qk_b)

Reassociation used on device (matmul associativity):
    v (q^T k) = (v q^T) k = M k = A x + c 1^T
      with  M = v q^T  [C,C],  A = M Wk,  c = M bk
    A^T = AT = (Wk^T Wq) U + (Wk^T bq) (x) v0sum + h (x) bv
      with  U = G WvT,  G = x x^T,  v0sum = Wv sx,  sx = x 1,
            h = (Wk^T Wq) sx + N Wk^T bq
    sum(qk_b) = (sum_n q) . (sum_m k)  — a dot of two C-vectors, computed on
    the host in float64 during the gather step (it is ~1e-5 of the FLOPs and
    the global S couples all samples anyway).

The device does the heavy work in fp32r (TF32-like, 11-bit mantissa, full PE
rate) with fp32 PSUM accumulation: G (x x^T via a host-pretransposed xT pack),
the [C,C] chain U -> AT (and M -> c), and the output matmul A x + c.
Host pre-rounds all fp32r operands to the tf32 grid (round-to-nearest-even,
11 explicit mantissa bits) so plain HWDGE DMAs can be used.

Everything streams: xT first (both HWDGE rings, chunked so G paces with the
DMA), then the weight pack, then x in column chunks (the output matmul paces
with those), with tiny vectors on the SWDGE queue.
"""
import sys
sys.path.insert(0, '/opt/trn_rl_repo')
from contextlib import ExitStack

import numpy as np

import concourse.bass as bass
from concourse import bacc
import concourse.mybir as mybir
import concourse.tile as tile
from concourse.bass_utils import run_bass_kernel_spmd

dt = mybir.dt
B, C, N = 8, 256, 2048
NB = N // 128
F32 = dt.float32
F32R = dt.float32r
Ident = mybir.ActivationFunctionType.Identity

_NC_CACHE = {}


def _round_tf32(a):
    """Round-to-nearest-even onto the fp32r (tf32-like) grid: keep 11 explicit
    mantissa bits (drop the low 12 of the fp32 mantissa)."""
    a = np.ascontiguousarray(a, np.float32)
    u = a.view(np.uint32).astype(np.uint64)
    u = (u + 0x7FF + ((u >> 12) & 1)) & 0xFFFFF000
    return u.astype(np.uint32).view(np.float32)


def _build(warmup_mms=2, warm_n=256):
    nc = bacc.Bacc("TRN2", target_bir_lowering=False, debug=False)

    xT = nc.declare_dram_parameter("xT", [128, 4096], F32R, isOutput=False)
    x_r = nc.declare_dram_parameter("x_r", [C, N], F32R, isOutput=False)
    w3r = nc.declare_dram_parameter("w3r", [C, 768], F32R, isOutput=False)
    evc = nc.declare_dram_parameter("evc", [128, 8], F32R, isOutput=False)
    evw = nc.declare_dram_parameter("evw", [1, 768], F32R, isOutput=False)
    attn = nc.declare_dram_parameter("attn", [C, N], F32, isOutput=True)

    with tile.TileContext(nc) as tc, ExitStack() as ctx:
        consts = ctx.enter_context(tc.tile_pool(name="consts", bufs=1))
        xpool = ctx.enter_context(tc.tile_pool(name="xpool", bufs=1))
        small = ctx.enter_context(tc.tile_pool(name="small", bufs=1))
        outp = ctx.enter_context(tc.tile_pool(name="outp", bufs=8))
        ps_big = ctx.enter_context(tc.tile_pool(name="ps_big", bufs=4, space="PSUM"))
        ps_g = ctx.enter_context(tc.tile_pool(name="ps_g", bufs=1, space="PSUM"))
        ps_sm = ctx.enter_context(tc.tile_pool(name="ps_sm", bufs=2, space="PSUM"))

        # PE warmup on a memset tile (no DMA dependency): keeps the HAM clock
        # busy so the real matmuls start at full rate.
        t_warm = consts.tile([128, warm_n], F32, name="warm")
        nc.vector.memset(t_warm[:], 0.5)
        for wi in range(warmup_mms):
            pw = ps_big.tile([128, warm_n], F32, name="warm_ps", tag="big")
            nc.tensor.matmul(pw[:], t_warm[:, 0:128], t_warm[:],
                             start=True, stop=True)

        # tiny vectors ride the SWDGE queue so the HWDGE rings stay clean
        t_evc = consts.tile([128, 8], F32R, name="evc")
        nc.gpsimd.dma_start(t_evc[:], evc[:])
        t_evw = consts.tile([1, 768], F32R, name="evw")
        nc.gpsimd.dma_start(t_evw[:], evw[:])

        t_xT = xpool.tile([128, 4096], F32R, name="xTp")
        t_x = [xpool.tile([128, N], F32R, name=f"x{i}") for i in range(2)]
        t_w3r = [consts.tile([128, 768], F32R, name=f"w3r{i}") for i in range(2)]

        for c in range(8):  # xT chunks alternate the two HWDGE rings
            csl = slice(c * 512, (c + 1) * 512)
            eng = nc.sync if c % 2 == 0 else nc.scalar
            eng.dma_start(t_xT[:, csl], xT[:, csl])
        nc.sync.dma_start(t_w3r[0][:], w3r[0:128, :])
        nc.scalar.dma_start(t_w3r[1][:], w3r[128:256, :])
        for mc in range(4):  # x column chunks (output matmul paces with these)
            msl = slice(mc * 512, (mc + 1) * 512)
            nc.sync.dma_start(t_x[0][:, msl], x_r[0:128, msl])
            nc.scalar.dma_start(t_x[1][:, msl], x_r[128:256, msl])

        t_wqT = [t_w3r[i][:, 0:256] for i in range(2)]
        t_wvT = [t_w3r[i][:, 256:512] for i in range(2)]
        t_QK = [t_w3r[i][:, 512:768] for i in range(2)]
        t_sx_r = [t_evc[:, 4 * i:4 * i + 2] for i in range(2)]
        t_bk2_col = [t_evc[:, 4 * i + 2:4 * i + 4] for i in range(2)]
        t_bq_row = t_evw[0:1, 0:256]
        t_bv_row = t_evw[0:1, 256:512]
        t_u1_row = t_evw[0:1, 512:768]

        # G = x x^T accumulated from the transposed pack (stream-paced)
        t_G_ps = [ps_g.tile([128, C], F32, name=f"G{ch}", tag=f"G{ch}")
                  for ch in range(2)]
        for nb in range(NB):
            xt = t_xT[:, nb * 256:(nb + 1) * 256]
            for ch in range(2):
                nc.tensor.matmul(t_G_ps[ch][:], xt[:, ch * 128:(ch + 1) * 128],
                                 xt, start=(nb == 0), stop=(nb == NB - 1))
        t_G = [small.tile([128, C], F32R, name=f"Gsb{ch}") for ch in range(2)]
        for ch in range(2):
            nc.vector.tensor_copy(t_G[ch][:], t_G_ps[ch][:])

        # row sums for the rank-1 bias folds; h for AT
        p_q0row = ps_sm.tile([1, C], F32, name="q0row", tag="sm")
        p_v0row = ps_sm.tile([1, C], F32, name="v0row", tag="sm")
        for kt in range(2):
            nc.tensor.matmul(p_q0row[:], t_sx_r[kt][:, 0:1], t_wqT[kt],
                             start=(kt == 0), stop=(kt == 1))
        for kt in range(2):
            nc.tensor.matmul(p_v0row[:], t_sx_r[kt][:, 0:1], t_wvT[kt],
                             start=(kt == 0), stop=(kt == 1))
        t_qsumf_row = small.tile([1, C], F32R, name="qsumf_row")
        t_bqN_row = small.tile([1, C], F32, name="bqN_row")
        nc.vector.tensor_scalar_mul(t_bqN_row[:], t_bq_row.bitcast(F32), float(N))
        nc.vector.tensor_add(t_qsumf_row[:], p_q0row[:], t_bqN_row[:])
        t_v0row = small.tile([1, C], F32R, name="v0row_sb")
        nc.vector.tensor_copy(t_v0row[:], p_v0row[:])

        p_hrow = ps_sm.tile([1, C], F32, name="hrow", tag="sm")
        for kt in range(2):
            nc.tensor.matmul(p_hrow[:], t_sx_r[kt][:, 0:1], t_QK[kt],
                             start=(kt == 0), stop=(kt == 1))
        t_u1N_row = small.tile([1, C], F32, name="u1N_row")
        nc.vector.tensor_scalar_mul(t_u1N_row[:], t_u1_row.bitcast(F32), float(N))
        t_h_row = small.tile([1, C], F32R, name="h_row")
        nc.vector.tensor_add(t_h_row[:], p_hrow[:], t_u1N_row[:])

        # U = G WvT ; AT = QK^T U + rank-1 folds (critical path)
        t_U_ps = [ps_sm.tile([128, C], F32, name=f"U_ps{ch}", tag="sm")
                  for ch in range(2)]
        for ch in range(2):
            for kt in range(2):
                nc.tensor.matmul(t_U_ps[ch][:], t_G[kt][:, ch * 128:(ch + 1) * 128],
                                 t_wvT[kt], start=(kt == 0), stop=(kt == 1))
        t_U = [small.tile([128, C], F32R, name=f"U{ch}") for ch in range(2)]
        for ch in range(2):
            nc.vector.tensor_copy(t_U[ch][:], t_U_ps[ch][:])

        t_AT_ps = [ps_sm.tile([128, C], F32, name=f"AT_ps{ch}", tag="sm")
                   for ch in range(2)]
        t_AT = [small.tile([128, C], F32R, name=f"AT{ch}") for ch in range(2)]
        for ch in range(2):
            csl = slice(ch * 128, (ch + 1) * 128)
            for kt in range(2):
                nc.tensor.matmul(t_AT_ps[ch][:], t_QK[kt][:, csl], t_U[kt][:],
                                 start=(kt == 0), stop=False)
            nc.tensor.matmul(t_AT_ps[ch][:], t_u1_row[:, csl], t_v0row[:],
                             start=False, stop=False)
            nc.tensor.matmul(t_AT_ps[ch][:], t_h_row[:, csl], t_bv_row[:],
                             start=False, stop=True)
            nc.vector.tensor_copy(t_AT[ch][:], t_AT_ps[ch][:])

        # MT = Wq U + rank-1 folds ; c = MT^T bk  (off the critical path)
        t_MT_ps = [ps_sm.tile([128, C], F32, name=f"MT_ps{ch}", tag="sm")
                   for ch in range(2)]
        for ch in range(2):
            csl = slice(ch * 128, (ch + 1) * 128)
            for kt in range(2):
                nc.tensor.matmul(t_MT_ps[ch][:], t_wqT[kt][:, csl], t_U[kt][:],
                                 start=(kt == 0), stop=False)
            nc.tensor.matmul(t_MT_ps[ch][:], t_bq_row[:, csl], t_v0row[:],
                             start=False, stop=False)
            nc.tensor.matmul(t_MT_ps[ch][:], t_qsumf_row[:, csl], t_bv_row[:],
                             start=False, stop=True)
        t_MT = [small.tile([128, C], F32R, name=f"MT{ch}") for ch in range(2)]
        for ch in range(2):
            nc.vector.tensor_copy(t_MT[ch][:], t_MT_ps[ch][:])
        t_c_ps = [ps_sm.tile([128, 2], F32, name=f"c_ps{ch}", tag="sm")
                  for ch in range(2)]
        t_c = [small.tile([128, 1], F32, name=f"c{ch}") for ch in range(2)]
        for ch in range(2):
            csl = slice(ch * 128, (ch + 1) * 128)
            for kt in range(2):
                nc.tensor.matmul(t_c_ps[ch][:], t_MT[kt][:, csl], t_bk2_col[kt],
                                 start=(kt == 0), stop=(kt == 1))
            nc.vector.tensor_copy(t_c[ch][:], t_c_ps[ch][:, 0:1])

        # attn = AT^T x + c, per x column chunk; outputs alternate rings
        for mc in range(4):
            msl = slice(mc * 512, (mc + 1) * 512)
            for ch in range(2):
                chsl = slice(ch * 128, (ch + 1) * 128)
                pa = ps_big.tile([128, 512], F32, name=f"attn_ps{mc}_{ch}",
                                 tag="big")
                for kt in range(2):
                    nc.tensor.matmul(pa[:], t_AT[kt][:, chsl], t_x[kt][:, msl],
                                     start=(kt == 0), stop=(kt == 1))
                ta = outp.tile([128, 512], F32, name=f"attn_sb{mc}_{ch}",
                               tag="attn_sb")
                if ch == 0:
                    nc.vector.tensor_scalar_add(ta[:], pa[:], t_c[ch][:])
                    nc.sync.dma_start(attn[chsl, msl], ta[:])
                else:
                    nc.scalar.activation(ta[:], pa[:], Ident,
                                         bias=t_c[ch][:], scale=1.0)
                    nc.scalar.dma_start(attn[chsl, msl], ta[:])

    nc.finalize()
    return nc


def _get_nc():
    if "nc" not in _NC_CACHE:
        _NC_CACHE["nc"] = _build()
    return _NC_CACHE["nc"]


def kernel(x, Wq, bq, Wk, bk, Wv, bv):
    x = np.ascontiguousarray(x, np.float32)
    Wq = np.ascontiguousarray(Wq, np.float32)
    Wk = np.ascontiguousarray(Wk, np.float32)
    Wv = np.ascontiguousarray(Wv, np.float32)
    bq = np.ascontiguousarray(bq, np.float32)
    bk = np.ascontiguousarray(bk, np.float32)
    bv = np.ascontiguousarray(bv, np.float32)
    assert x.shape == (B, C, N), x.shape

    nc = _get_nc()

    # shared host prep
    QK = (Wq.T.astype(np.float64) @ Wk.astype(np.float64)).astype(np.float32)
    u1 = (Wk.T.astype(np.float64) @ bq.astype(np.float64)).astype(np.float32)
    w3r = _round_tf32(np.concatenate([Wq.T, Wv.T, QK], axis=1))
    evw = _round_tf32(np.concatenate([bq, bv, u1])[None, :])

    ins = []
    s_host = []
    for b in range(B):
        sx64 = x[b].astype(np.float64).sum(axis=1)
        sx = sx64.astype(np.float32)
        # per-sample global qk sum: (sum_n q) . (sum_m k), exact in float64
        qsum = Wq.astype(np.float64) @ sx64 + N * bq.astype(np.float64)
        ksum = Wk.astype(np.float64) @ sx64 + N * bk.astype(np.float64)
        s_host.append(float(qsum @ ksum))
        xr = _round_tf32(x[b])
        xTp = np.ascontiguousarray(
            xr.T.reshape(16, 128, 256).transpose(1, 0, 2).reshape(128, 4096))
        evc = np.zeros((128, 8), np.float32)
        for i in range(2):
            sl = slice(i * 128, (i + 1) * 128)
            evc[:, 4 * i] = sx[sl]
            evc[:, 4 * i + 2] = bk[sl]
        evc = _round_tf32(evc)
        ins.append(dict(x_r=xr, xT=xTp, w3r=w3r, evc=evc, evw=evw))

    res = run_bass_kernel_spmd(nc, ins, list(range(B)))

    S = np.float64(np.sum(s_host))
    out = np.stack([res.results[b]["attn"] for b in range(B)])
    return (out * np.float32(1.0 / S)).astype(np.float32)


if __name__ == "__main__":
    rng = np.random.default_rng(0)
    inputs = {
        "x": rng.standard_normal((B, C, N), dtype=np.float32),
        "Wq": rng.standard_normal((C, C), dtype=np.float32) / 16,
        "bq": rng.standard_normal(C, dtype=np.float32) / 16,
        "Wk": rng.standard_normal((C, C), dtype=np.float32) / 16,
        "bk": rng.standard_normal(C, dtype=np.float32) / 16,
        "Wv": rng.standard_normal((C, C), dtype=np.float32) / 16,
        "bv": rng.standard_normal(C, dtype=np.float32) / 16,
    }
    out = kernel(**inputs)
    print("kernel output:", out.shape, out.dtype, float(np.abs(out).max()))
